# revision 38
# baseline (speedup 1.0000x reference)
"""AttentionBlock (GroupNorm -> qkv 1x1 -> 4-head attention over 4096 tokens
-> proj 1x1 -> residual) distributed over 8 TRN2 NeuronCores.

Sharding: zero-communication query sharding. Core j handles batch b = j//2 and
query half qh = j%2 (2048 of the 4096 spatial positions). Each core loads the
full x[b] (256, 4096), computes GroupNorm + K/V over all keys, Q only for its
2048 queries, and writes its (256, 2048) output slice.

Structure (bf16 PE shapes -- measured on HW: fp8/DoubleRow gives no PE gain;
512-col bf16 matmuls and 65-col AV matmuls with hidden weight loads are the
throughput-optimal shapes; 64-partition matmul pairs at row quadrants 0/64
execute concurrently on the PE):
  - scores transposed ([keys, queries], lhsT=k rhs=q) so exp output feeds AV
    directly; denominator rides as a ones column in V^T; per-partition
    reciprocal normalization (no cross-partition broadcasts).
  - K stored per head-PAIR [2x64ch, keys] (real 64-deep contract, base
    partition 0/64): halves K production vs zero-padded per-head tiles, and
    the two heads' QK matmuls run in parallel on disjoint PE row-quadrants.
  - exp tiles split ~60/40 between the Act engine (native Exp) and the DVE
    (one tensor_scalar: bits = A16*scale*s + B16 -> int16 = bf16 pattern of
    e^s, Schraudolph; rel-err ~2%, damped ~30x by the residual). Only these
    two engines can read PSUM, so they bound softmax throughput.
  - 3 score buffers (6 psum banks) + 2 oT accumulators (2 banks); transposes,
    proj and K/V/Q production psums time-share the score pool. AV pairs trail
    QK/exp by LAGP slots so the in-order PE stream never waits on a fresh exp.
  - GroupNorm stats aggregate via bf16 mask matmuls with the variance
    recombination (E[var]+E[mean^2]-mean^2) done post-aggregation; rsqrt via
    the fp32 bit-trick + Newton (no Act table load).
  - queries/residual path ships as bf16 (xq); end-to-end rel err ~1.9e-3
    vs the 2e-2 gate. Measured 256 us (baseline 396/332 us).
"""

import numpy as np

import concourse.bass as bass
import concourse.tile as tile
from concourse import bacc, mybir
from concourse.bass_utils import run_bass_kernel_spmd

C = 256
HW = 4096
NH = 4
D = 64  # head dim
G = 8  # groups
EPS = 1e-5
SCALE = D**-0.5
Q = HW // 2  # queries per core
NJT = HW // 128  # 32 key tiles
NKC = 8  # key chunks (512 keys each) for K/V production
NIC = Q // 512  # 4 query chunks of 512

F32 = mybir.dt.float32
BF16 = mybir.dt.bfloat16
I16 = mybir.dt.int16

# one-op exp on the DVE: exp(t) ~= bitcast_bf16(int16(A16*t + B16)) (Schraudolph
# in bf16 bit space; C=5.5 minimizes rms rel err ~1.8%, +0.5 compensates the
# truncating float->int convert).
A16 = 128.0 / float(np.log(2.0))
B16 = 127.0 * 128.0 - 5.5 + 0.5
LAGP = 4  # AV pairs trail QK/exp by this many jp slots


def _route_pattern(na, nv, n=64):
    """Largest-remainder interleave of n exp tiles across (act, dve)."""
    quota = {"a": na, "v": nv}
    cnt = {"a": 0, "v": 0}
    out = []
    for r in range(n):
        e = max(("a", "v"), key=lambda k: quota[k] * (r + 1) / n - cnt[k])
        cnt[e] += 1
        out.append(e)
    return out


def _hole_pattern(nv):
    """32-tile block routing with tiles 6..11 pinned to Act: the deferred
    normalize finisher lands there in the DVE stream (head-of-line)."""
    slots = list(range(0, 6)) + list(range(12, 32))
    out = ["a"] * 32
    for k in range(nv):
        out[slots[(k * len(slots)) // nv]] = "v"
    return out


def build(finalize=True):
    nc = bacc.Bacc("TRN2", target_bir_lowering=False, debug=False, num_devices=8)

    x = nc.declare_dram_parameter("x", [C, HW], BF16, isOutput=False)
    xq = nc.declare_dram_parameter("xq", [C, Q], BF16, isOutput=False)
    wn2 = nc.declare_dram_parameter("wn2", [128, 2], F32, isOutput=False)
    bn2 = nc.declare_dram_parameter("bn2", [128, 2], F32, isOutput=False)
    wq = nc.declare_dram_parameter("wq", [128, 2, C], BF16, isOutput=False)
    bq2 = nc.declare_dram_parameter("bq2", [128, 2], F32, isOutput=False)
    wk = nc.declare_dram_parameter("wk", [128, 2, C], BF16, isOutput=False)
    bk2 = nc.declare_dram_parameter("bk2", [128, 2], F32, isOutput=False)
    wv = nc.declare_dram_parameter("wv", [128, 2, NH * 65], BF16, isOutput=False)
    vb = nc.declare_dram_parameter("vb", [128, NH * 65], F32, isOutput=False)
    wproj = nc.declare_dram_parameter("wproj", [128, 2, C], BF16, isOutput=False)
    ident = nc.declare_dram_parameter("ident", [128, 128], BF16, isOutput=False)
    bproj2 = nc.declare_dram_parameter("bproj2", [128, 2], F32, isOutput=False)
    gmask = nc.declare_dram_parameter("gmask", [128, 2, 128], BF16, isOutput=False)
    gmaskT = nc.declare_dram_parameter("gmaskT", [128, 2, 128], BF16, isOutput=False)
    out = nc.declare_dram_parameter("out", [C, Q], F32, isOutput=True)

    Exp = mybir.ActivationFunctionType.Exp
    Alu = mybir.AluOpType

    with tile.TileContext(nc) as tc:
        with (
            tc.tile_pool(name="keep", bufs=1) as keep,
            tc.tile_pool(name="consts", bufs=1) as consts,
            tc.tile_pool(name="small", bufs=4) as small,
            tc.tile_pool(name="s_ps", bufs=3, space="PSUM") as s_ps,
            tc.tile_pool(name="acc_ps", bufs=1, space="PSUM") as acc_ps,
            tc.tile_pool(name="exps", bufs=2 * (LAGP + 1)) as expp,
            tc.tile_pool(name="att", bufs=2) as att,
        ):
            # persistent attention operands
            # K per head-pair: rows 0..63 = head 2hp, 64..127 = head 2hp+1
            KZ = [
                keep.tile([128, HW], BF16, tag=f"KZ{t}", name=f"KZ{t}")
                for t in range(2)
            ]
            QT = [
                keep.tile([128, Q], BF16, tag=f"Q{t}", name=f"Q{t}")
                for t in range(2)
            ]
            # V^T with a leading ones column per head: [keys, (head, 1+d)]
            V = keep.tile([128, NJT, NH * 65], BF16)
            XQ = [
                keep.tile([128, Q], BF16, tag=f"XQ{t}", name=f"XQ{t}")
                for t in range(2)
            ]

            with tc.tile_pool(name="xh", bufs=1) as xh:
                X = [
                    xh.tile([128, HW], BF16, tag=f"X{t}", name=f"X{t}")
                    for t in range(2)
                ]
                H = [
                    xh.tile([128, HW], BF16, tag=f"H{t}", name=f"H{t}")
                    for t in range(2)
                ]
                HQ = [
                    xh.tile([128, Q], BF16, tag=f"HQ{t}", name=f"HQ{t}")
                    for t in range(2)
                ]

                # preload the Exp activation table while DMAs run, and wake
                # the gpsimd firmware so its first real op pays no launch cost
                tldum = small.tile([1, 1], F32, tag="tld", name="tld", bufs=1)
                nc.vector.memset(tldum, 1.0)
                nc.scalar.activation(out=tldum, in_=tldum, func=Exp)
                gpdum = small.tile([1, 1], F32, tag="gpd", name="gpd", bufs=1)
                nc.gpsimd.memset(gpdum, 0.0)

                # ---- x DMA in chunks, bn_stats per chunk ----
                st = [
                    small.tile([128, 8, 6], F32, tag=f"bnst{t}", name=f"bnst{t}")
                    for t in range(2)
                ]
                JW = small.tile([128, 128], BF16, tag="junkw", name="junkw", bufs=1)
                JR = small.tile([128, 512], BF16, tag="junkr", name="junkr", bufs=1)
                nc.vector.memset(JW, 0.0)
                for ch in range(4):
                    for t in range(2):
                        eng = nc.sync if t == 0 else nc.scalar
                        eng.dma_start(
                            out=X[t][:, ch * 1024 : (ch + 1) * 1024],
                            in_=x[t * 128 : (t + 1) * 128, ch * 1024 : (ch + 1) * 1024],
                        )
                        xr = X[t].rearrange("p (n f) -> p n f", f=512)
                        for s in (2 * ch, 2 * ch + 1):
                            nc.vector.bn_stats(out=st[t][:, s], in_=xr[:, s])
                    if ch == 3:
                        # memset lands here in the DVE stream: the PE warmup
                        # below starts as stats wind down, so the array is
                        # still at full clock when the real matmuls arrive
                        nc.vector.memset(JR, 0.0)
                wps = s_ps.tile([128, 1024], F32, tag="sps", name="wps")
                for w in range(6):
                    nc.tensor.matmul(
                        out=wps[:, 0:512], lhsT=JW, rhs=JR, start=True, stop=True
                    )
                # ---- weights (after the x chunks in queue order; the
                # small GroupNorm masks go FIRST -- the stats matmul needs
                # them right after bn_aggr, before xq) ----
                GM = consts.tile([128, 2, 128], BF16)
                nc.sync.dma_start(out=GM, in_=gmask[:])
                GMT = consts.tile([128, 2, 128], BF16)
                nc.scalar.dma_start(out=GMT, in_=gmaskT[:])
                WN = consts.tile([128, 2], F32)
                nc.sync.dma_start(out=WN, in_=wn2[:])
                BN = consts.tile([128, 2], F32)
                nc.scalar.dma_start(out=BN, in_=bn2[:])
                for t in range(2):
                    eng = nc.sync if t == 0 else nc.scalar
                    eng.dma_start(out=XQ[t], in_=xq[t * 128 : (t + 1) * 128, :])
                WQ = consts.tile([128, 2, C], BF16)
                nc.sync.dma_start(out=WQ, in_=wq[:])
                BQ = consts.tile([128, 2], F32)
                nc.scalar.dma_start(out=BQ, in_=bq2[:])
                WK = consts.tile([128, 2, C], BF16)
                nc.sync.dma_start(out=WK, in_=wk[:])
                BK = consts.tile([128, 2], F32)
                nc.scalar.dma_start(out=BK, in_=bk2[:])
                WV = consts.tile([128, 2, NH * 65], BF16)
                nc.sync.dma_start(out=WV, in_=wv[:])
                VB = consts.tile([128, NH * 65], F32)
                nc.scalar.dma_start(out=VB, in_=vb[:])
                WP = consts.tile([128, 2, C], BF16)
                nc.sync.dma_start(out=WP, in_=wproj[:])
                IDENT = consts.tile([128, 128], BF16)
                nc.scalar.dma_start(out=IDENT, in_=ident[:])
                BP = consts.tile([128, 2], F32)
                nc.scalar.dma_start(out=BP, in_=bproj2[:])

                # ---- GroupNorm statistics ----
                # bf16 stats operands keep the aggregation matmuls off the
                # slow fp32 PE path; precision impact ~0.2% on rstd.
                mv3 = small.tile([128, 2, 3], BF16)  # [:, t, (mean, var, mean^2)]
                for t in range(2):
                    mv = small.tile([128, 2], F32, tag="bnmv")
                    nc.vector.bn_aggr(out=mv, in_=st[t])
                    nc.vector.tensor_copy(out=mv3[:, t, 0:2], in_=mv)
                    nc.vector.tensor_tensor(
                        out=mv3[:, t, 2:3], in0=mv[:, 0:1], in1=mv[:, 0:1],
                        op=Alu.mult,
                    )

                gps = s_ps.tile([128, 1024], F32, tag="sps", name="gnps")
                for t in range(2):
                    nc.tensor.matmul(
                        out=gps[:, 0:3], lhsT=GM[:, t], rhs=mv3[:, t],
                        start=(t == 0), stop=(t == 1),
                    )
                gsb = small.tile([128, 3], F32)
                nc.vector.tensor_copy(out=gsb, in_=gps[:, 0:3])
                # gstat rows 0..8: col0 = group mean, col1 = rsqrt(var);
                # rows 8..128 stay zero for the padded broadcast matmul.
                gstat = small.tile([128, 2], BF16)
                nc.vector.memset(gstat, 0.0)
                nc.vector.tensor_copy(out=gstat[:G, 0:1], in_=gsb[:G, 0:1])
                # var_g = E[var] + E[mean^2] - mean_g^2 (EPS=1e-5 negligible
                # at var ~ 1)
                gvar = small.tile([G, 1], F32)
                nc.vector.tensor_tensor(
                    out=gvar, in0=gsb[:G, 1:2], in1=gsb[:G, 2:3], op=Alu.add
                )
                gm2 = small.tile([G, 1], F32, tag="gm2", name="gm2")
                nc.vector.tensor_tensor(
                    out=gm2, in0=gsb[:G, 0:1], in1=gsb[:G, 0:1], op=Alu.mult
                )
                nc.vector.tensor_tensor(
                    out=gvar, in0=gvar, in1=gm2, op=Alu.subtract
                )
                # rsqrt via the fp32 bit-trick seed + Newton step (keeps the
                # Act engine free of Ln table loads)
                gvi = gvar.bitcast(mybir.dt.int32)
                y0i = small.tile([G, 1], mybir.dt.int32, tag="y0i", name="y0i")
                nc.vector.tensor_scalar(
                    out=y0i, in0=gvi, scalar1=1, scalar2=0,
                    op0=Alu.logical_shift_right, op1=Alu.bitwise_or,
                )
                nc.vector.tensor_scalar(
                    out=y0i, in0=y0i, scalar1=-1, scalar2=0x5F3759DF,
                    op0=Alu.mult, op1=Alu.add,
                )
                y = y0i.bitcast(F32)
                yt = small.tile([G, 1], F32, tag="yt", name="yt")
                for _ in range(1):
                    nc.vector.tensor_tensor(out=yt, in0=y, in1=y, op=Alu.mult)
                    nc.vector.tensor_tensor(out=yt, in0=yt, in1=gvar, op=Alu.mult)
                    nc.vector.tensor_scalar(
                        out=yt, in0=yt, scalar1=-0.5, scalar2=1.5,
                        op0=Alu.mult, op1=Alu.add,
                    )
                    nc.vector.tensor_tensor(out=y, in0=y, in1=yt, op=Alu.mult)
                nc.vector.tensor_copy(out=gstat[:G, 1:2], in_=y)

                # broadcast group stats back to channels
                AB = []  # [t] -> [128, 2] (alpha, beta)
                for t in range(2):
                    bc = s_ps.tile([128, 1024], F32, tag="sps", name="bcst")
                    nc.tensor.matmul(out=bc[:, 0:2], lhsT=GMT[:, t], rhs=gstat)
                    bsb = small.tile([128, 2], F32, tag="bsb", name="bsb")
                    nc.vector.tensor_copy(out=bsb, in_=bc[:, 0:2])
                    ab = small.tile([128, 2], F32, tag=f"ab{t}", name=f"ab{t}")
                    # alpha = rstd * w
                    nc.vector.tensor_tensor(
                        out=ab[:, 0:1], in0=bsb[:, 1:2], in1=WN[:, t : t + 1],
                        op=Alu.mult,
                    )
                    # beta = b - mean * alpha
                    nc.vector.tensor_tensor(
                        out=ab[:, 1:2], in0=bsb[:, 0:1], in1=ab[:, 0:1],
                        op=Alu.mult,
                    )
                    nc.vector.tensor_tensor(
                        out=ab[:, 1:2], in0=BN[:, t : t + 1], in1=ab[:, 1:2],
                        op=Alu.subtract,
                    )
                    AB.append(ab)

                # ---- chunked production helpers ----
                def hq_chunk(c, eng=None):  # normalized queries, 512 cols
                    for t in range(2):
                        (eng or nc.gpsimd).tensor_scalar(
                            out=HQ[t][:, c * 512 : (c + 1) * 512],
                            in0=XQ[t][:, c * 512 : (c + 1) * 512],
                            scalar1=AB[t][:, 0:1], scalar2=AB[t][:, 1:2],
                            op0=Alu.mult, op1=Alu.add,
                        )

                def h_chunk(c):  # normalized keys, 512 cols (gpsimd: pure
                    # SBUF->SBUF, keeps the DVE free for psum drains)
                    for t in range(2):
                        nc.gpsimd.tensor_scalar(
                            out=H[t][:, c * 512 : (c + 1) * 512],
                            in0=X[t][:, c * 512 : (c + 1) * 512],
                            scalar1=AB[t][:, 0:1], scalar2=AB[t][:, 1:2],
                            op0=Alu.mult, op1=Alu.add,
                        )

                def q_chunk(c):  # q projection for queries 512c.. (both t)
                    for t in range(2):
                        ps = s_ps.tile([128, 1024], F32, tag="sps", name="qps")
                        for ct in range(2):
                            nc.tensor.matmul(
                                out=ps[:, 0:512],
                                lhsT=WQ[:, ct, t * 128 : (t + 1) * 128],
                                rhs=HQ[ct][:, c * 512 : (c + 1) * 512],
                                start=(ct == 0), stop=(ct == 1),
                            )
                        # drain on Act: it has slack at production slots and
                        # the dep (the matmul just above) resolves quickly
                        nc.scalar.add(
                            QT[t][:, c * 512 : (c + 1) * 512],
                            ps[:, 0:512], BQ[:, t : t + 1],
                        )

                def k_piece(n, hp):  # K head-pair hp for keys 512n..
                    ps = s_ps.tile([128, 1024], F32, tag="sps", name="kps")
                    for ct in range(2):
                        nc.tensor.matmul(
                            out=ps[:, 0:512],
                            lhsT=WK[:, ct, hp * 128 : (hp + 1) * 128],
                            rhs=H[ct][:, n * 512 : (n + 1) * 512],
                            start=(ct == 0), stop=(ct == 1),
                        )
                    nc.scalar.add(
                        KZ[hp][:, n * 512 : (n + 1) * 512],
                        ps[:, 0:512], BK[:, hp : hp + 1],
                    )

                def v_piece(n, half):  # V^T for key tiles 4n+2*half(+1)
                    j0 = 4 * n + 2 * half
                    ps = s_ps.tile([128, 1024], F32, tag="sps", name="vps")
                    for jo in range(2):
                        for ct in range(2):
                            nc.tensor.matmul(
                                out=ps[:, jo * 512 : jo * 512 + NH * 65],
                                lhsT=H[ct][:, (j0 + jo) * 128 : (j0 + jo + 1) * 128],
                                rhs=WV[:, ct],
                                start=(ct == 0), stop=(ct == 1),
                            )
                        nc.vector.tensor_tensor(
                            out=V[:, j0 + jo],
                            in0=ps[:, jo * 512 : jo * 512 + NH * 65],
                            in1=VB, op=Alu.add,
                        )

                # minimal chain to the first QK: chunk 0 of HQ/H/Q/K.
                # h(0) on gpsimd and hq(0) on the DVE run in parallel.
                h_chunk(0)
                hq_chunk(0, nc.vector)
                k_piece(0, 0)
                q_chunk(0)

                # ic0 production schedule: jp slot -> tasks, per hp.
                # V chunk m lands at slot 2m (just in time for its own AVs),
                # K chunk m+1 and H chunk m+2 at slot 2m+1; hp1 only needs
                # its own K head-pair. q/hq chunks ride along for later ics.
                prod0, prod1 = {}, {}
                prod0[0] = [
                    lambda: h_chunk(1),
                    lambda: v_piece(0, 0), lambda: v_piece(0, 1),
                ]
                for m in range(1, NKC):
                    tasks = []
                    if m + 1 < NKC:
                        tasks.append(lambda c=m + 1: h_chunk(c))
                    tasks.append(lambda c=m: k_piece(c, 0))
                    prod0[2 * m - 1] = tasks
                    prod0[2 * m] = [
                        lambda c=m: v_piece(c, 0), lambda c=m: v_piece(c, 1)
                    ]

                prod0[14] = prod0.get(14, []) + [lambda: k_piece(0, 1)]
                for m in range(1, NKC):
                    prod1[2 * m - 1] = [lambda c=m: k_piece(c, 1)]

                # exp tile routing across THREE consumers: Act (native Exp),
                # DVE (one-op Schraudolph from psum), gpsimd (Schraudolph from
                # a DMA-staged SBUF copy -- gpsimd cannot read psum; the copy
                # rides the idle sync DMA queue). ic0 leans on Act (DVE does
                # production drains there, gpsimd the h chunks).
                exp_cnt = [0]
                PAT0 = _route_pattern(48, 16)
                PAT = _route_pattern(37, 27)

                def do_exp(S, E):
                    i = exp_cnt[0]
                    exp_cnt[0] += 1
                    lab = (PAT0 if i < 64 else PAT)[i % 64]
                    if lab == "v":
                        nc.vector.tensor_scalar(
                            out=E, in0=S, scalar1=A16 * SCALE, scalar2=B16,
                            op0=Alu.mult, op1=Alu.add,
                        )
                    else:
                        nc.scalar.activation(
                            out=E.bitcast(BF16), in_=S, func=Exp, scale=SCALE
                        )

                # ---- attention + projection (oT form: queries on psum
                # partitions, exp(scores) streamed as the stationary operand,
                # per-partition softmax normalization).
                # AV matmuls and per-block finishers (normalize, transpose,
                # proj, residual) are DEFERRED through a global work queue so
                # the next block's QK/exp stream is emitted ahead of them --
                # the in-order engines never serialize at hp/ic boundaries.
                pend = []      # (emit_av_closure, block_key)
                finishers = {}  # block_key -> closure run after its last AV

                def pump(k):
                    for _ in range(k):
                        if not pend:
                            break
                        fn, key = pend.pop(0)
                        fn()
                        if key in finishers and not any(
                            k2 == key for _, k2 in pend
                        ):
                            finishers.pop(key)()

                for ic in range(NIC):
                    oTn = att.tile([128, 4, 256], BF16, tag="oTn", name="oTn")
                    OSB = att.tile([128, 2, 512], BF16, tag="osb", name="osb")
                    for hp in range(2):
                        oT = [
                            acc_ps.tile(
                                [128, 4, 68], F32, tag=f"ot{h2}", name=f"ot{h2}"
                            )
                            for h2 in range(2)
                        ]

                        def qk_into(S, j, hp=hp, ic=ic):
                            for h2 in range(2):
                                nc.tensor.matmul(
                                    out=S[:, h2 * 512 : (h2 + 1) * 512],
                                    lhsT=KZ[hp][
                                        h2 * 64 : (h2 + 1) * 64,
                                        j * 128 : (j + 1) * 128,
                                    ],
                                    rhs=QT[hp][
                                        h2 * 64 : (h2 + 1) * 64,
                                        ic * 512 : (ic + 1) * 512,
                                    ],
                                    start=True, stop=True,
                                )

                        def av_from(E, j, oT=oT, hp=hp):
                            for h2 in range(2):
                                head = 2 * hp + h2
                                for isub in range(4):
                                    nc.tensor.matmul(
                                        out=oT[h2][:, isub, 0:65],
                                        lhsT=E[
                                            :,
                                            h2 * 512 + isub * 128 : h2 * 512
                                            + (isub + 1) * 128,
                                        ].bitcast(BF16),
                                        rhs=V[:, j, head * 65 : (head + 1) * 65],
                                        start=(j == 0 and isub == 0),
                                        stop=(j == NJT - 1 and isub == 3),
                                    )

                        for jp in range(NJT // 2):
                            if ic == 0:
                                sched = prod0 if hp == 0 else prod1
                                for task in sched.get(jp, ()):
                                    task()
                            # next ic's Q production rides the tail of this
                            # ic (ic0 is congested with K/V production)
                            if hp == 1 and jp == 13 and ic < NIC - 1:
                                hq_chunk(ic + 1)
                                q_chunk(ic + 1)
                            for jo in range(2):
                                j = 2 * jp + jo
                                S = s_ps.tile(
                                    [128, 1024], F32, tag="sps", name="s"
                                )
                                qk_into(S, j)
                                E = expp.tile(
                                    [128, 1024], I16, tag="exps", name="e"
                                )
                                do_exp(S, E)
                                pend.append(
                                    (lambda E=E, j=j, f=av_from: f(E, j), (ic, hp))
                                )
                            if len(pend) > 2 * LAGP:
                                pump(2)

                        def mk_norm(oT=oT, hp=hp, oTn=oTn):
                            def fin():
                                # normalize by the ones-column sums (per-
                                # partition; one strided reciprocal covers all
                                # 4 sub-tiles)
                                for h2 in range(2):
                                    head = 2 * hp + h2
                                    r4 = small.tile(
                                        [128, 4], F32, tag="recip", name="recip"
                                    )
                                    nc.vector.reciprocal(
                                        out=r4,
                                        in_=oT[h2][:, :, 64:65].rearrange(
                                            "p a b -> p (a b)"
                                        ),
                                    )
                                    for isub in range(4):
                                        nc.vector.tensor_scalar_mul(
                                            out=oTn[
                                                :, isub, head * 64 : (head + 1) * 64
                                            ],
                                            in0=oT[h2][:, isub, 0:64],
                                            scalar1=r4[:, isub : isub + 1],
                                        )
                            return fin

                        finishers[(ic, hp)] = mk_norm()

                    def mk_boundary(norm1=finishers[(ic, 1)], oTn=oTn, OSB=OSB, ic=ic):
                        def fin():
                            norm1()
                            # the whole boundary shares ONE score-pool tile
                            # (transposes in bank A as bf16, then proj reuses
                            # both banks) so only one rotation slot is held
                            bnd = s_ps.tile([128, 1024], F32, tag="sps", name="bnd")
                            tpb = bnd.bitcast(BF16)  # [128, 2048]
                            for ct in range(2):
                                for isub in range(4):
                                    k8 = ct * 4 + isub
                                    nc.tensor.matmul(
                                        tpb[:, k8 * 128 : (k8 + 1) * 128],
                                        oTn[:, isub, ct * 128 : (ct + 1) * 128],
                                        IDENT,
                                        is_transpose=True,
                                        start=(k8 == 0), stop=(k8 == 7),
                                    )
                            for ct in range(2):
                                nc.vector.tensor_copy(
                                    out=OSB[:, ct],
                                    in_=tpb[:, ct * 512 : (ct + 1) * 512],
                                )
                            # proj + bias + residual (reusing bnd)
                            pj = bnd
                            for mt in range(2):
                                for ct in range(2):
                                    nc.tensor.matmul(
                                        out=pj[:, mt * 512 : (mt + 1) * 512],
                                        lhsT=WP[:, ct, mt * 128 : (mt + 1) * 128],
                                        rhs=OSB[:, ct],
                                        start=(ct == 0), stop=(ct == 1),
                                    )
                                ob = att.tile(
                                    [128, 512], F32, tag="outsb", name="outsb"
                                )
                                nc.vector.scalar_tensor_tensor(
                                    out=ob, in0=pj[:, mt * 512 : (mt + 1) * 512],
                                    scalar=BP[:, mt : mt + 1],
                                    in1=XQ[mt][:, ic * 512 : (ic + 1) * 512],
                                    op0=Alu.add, op1=Alu.add,
                                )
                                nc.sync.dma_start(
                                    out=out[
                                        mt * 128 : (mt + 1) * 128,
                                        ic * 512 : (ic + 1) * 512,
                                    ],
                                    in_=ob,
                                )
                        return fin

                    finishers[(ic, 1)] = mk_boundary()
                pump(len(pend))
    if finalize:
        nc.finalize()
    return nc


def _prep_weights(norm_w, norm_b, qkv_w, qkv_b, proj_w, proj_b):
    """Host-side layout (pure reshapes/transposes + dtype casts of weights)."""
    import ml_dtypes

    f = np.float32
    cdt = ml_dtypes.bfloat16

    def ctile(v):  # (256,) -> (128, 2) per channel-tile columns
        return np.ascontiguousarray(np.asarray(v).reshape(2, 128).T, dtype=f)

    def ptile(m):  # (256, N) -> (128, 2, N)
        return np.ascontiguousarray(
            np.asarray(m).reshape(2, 128, -1).transpose(1, 0, 2), dtype=f
        )

    qkv_w = np.asarray(qkv_w)
    qkv_b = np.asarray(qkv_b)
    wqT = qkv_w[:C].T  # (256, 256)
    wkT = qkv_w[C : 2 * C].T  # (256, 256): out col o = head-pair*128 + row
    wvm = qkv_w[2 * C :]  # (256, 256)
    wvT = np.zeros((C, NH * 65), dtype=f)
    vb = np.zeros((128, NH * 65), dtype=f)
    for h in range(NH):
        wvT[:, h * 65 : h * 65 + 64] = wvm[h * 64 : (h + 1) * 64].T
        vb[:, h * 65 : h * 65 + 64] = qkv_b[
            2 * C + h * 64 : 2 * C + (h + 1) * 64
        ][None, :]
        vb[:, h * 65 + 64] = 1.0  # ones column -> denominator at oT column 64
    # zero-padded group masks (value 1/32 for group-mean aggregation; one-hot
    # transpose for the broadcast back to channels)
    gm = np.zeros((C, 128), dtype=f)
    for c in range(C):
        gm[c, c // 32] = 1.0 / 32.0
    gmaskT = np.zeros((128, 2, 128), dtype=f)
    for c in range(C):
        gmaskT[c // 32, c // 128, c % 128] = 1.0

    return dict(
        wn2=ctile(norm_w),
        bn2=ctile(norm_b),
        wq=ptile(wqT).astype(cdt),
        bq2=ctile(qkv_b[:C]),
        wk=ptile(wkT).astype(cdt),
        bk2=ctile(qkv_b[C : 2 * C]),
        wv=ptile(wvT).astype(cdt),
        vb=vb,
        wproj=ptile(np.asarray(proj_w).T).astype(cdt),
        ident=np.eye(128, dtype=cdt),
        bproj2=ctile(proj_b),
        gmask=ptile(gm).astype(cdt),
        gmaskT=gmaskT.astype(cdt),
    )


_NC_CACHE = {}
_RUN_OPTS = {}  # extra kwargs for run_bass_kernel_spmd (test harness sets trace)
LAST_RESULT = None


def _get_nc():
    if "nc" not in _NC_CACHE:
        _NC_CACHE["nc"] = build()
    return _NC_CACHE["nc"]


def kernel(x, norm_w, norm_b, qkv_w, qkv_b, proj_w, proj_b, **_):
    import ml_dtypes

    nc = _get_nc()
    w = _prep_weights(norm_w, norm_b, qkv_w, qkv_b, proj_w, proj_b)
    x = np.asarray(x, dtype=np.float32)
    Bv, Cv, Hv, Wv = x.shape
    xf = x.reshape(Bv, Cv, Hv * Wv)
    xb = xf.astype(ml_dtypes.bfloat16)
    in_maps = []
    for j in range(8):
        b, qh = j // 2, j % 2
        m = dict(w)
        m["x"] = np.ascontiguousarray(xb[b])
        m["xq"] = np.ascontiguousarray(xb[b][:, qh * Q : (qh + 1) * Q])
        in_maps.append(m)
    res = run_bass_kernel_spmd(nc, in_maps, core_ids=list(range(8)), **_RUN_OPTS)
    global LAST_RESULT
    LAST_RESULT = res
    outf = np.empty((Bv, Cv, Hv * Wv), dtype=np.float32)
    for j in range(8):
        b, qh = j // 2, j % 2
        outf[b][:, qh * Q : (qh + 1) * Q] = res.results[j]["out"]
    return outf.reshape(Bv, Cv, Hv, Wv)


# revision 39
# speedup vs baseline: 1.2179x; 1.2179x over previous
"""AttentionBlock (GroupNorm -> qkv 1x1 -> 4-head attention over 4096 tokens
-> proj 1x1 -> residual) distributed over 8 TRN2 NeuronCores.

Sharding: zero-communication query sharding. Core j handles batch b = j//2 and
query half qh = j%2 (2048 of the 4096 spatial positions). Each core loads the
full x[b] (256, 4096), computes GroupNorm + K/V over all keys, Q only for its
2048 queries, and writes its (256, 2048) output slice.

Structure (bf16 PE shapes -- measured on HW: fp8/DoubleRow gives no PE gain;
512-col bf16 matmuls and 65-col AV matmuls with hidden weight loads are the
throughput-optimal shapes; 64-partition matmul pairs at row quadrants 0/64
execute concurrently on the PE):
  - scores transposed ([keys, queries], lhsT=k rhs=q) so exp output feeds AV
    directly; denominator rides as a ones column in V^T; per-partition
    reciprocal normalization (no cross-partition broadcasts).
  - K stored per head-PAIR [2x64ch, keys] (real 64-deep contract, base
    partition 0/64): halves K production vs zero-padded per-head tiles, and
    the two heads' QK matmuls run in parallel on disjoint PE row-quadrants.
  - exp tiles split ~60/40 between the Act engine (native Exp) and the DVE
    (one tensor_scalar: bits = A16*scale*s + B16 -> int16 = bf16 pattern of
    e^s, Schraudolph; rel-err ~2%, damped ~30x by the residual). Only these
    two engines can read PSUM, so they bound softmax throughput.
  - 3 score buffers (6 psum banks) + 2 oT accumulators (2 banks); transposes,
    proj and K/V/Q production psums time-share the score pool. AV pairs trail
    QK/exp by LAGP slots so the in-order PE stream never waits on a fresh exp.
  - GroupNorm stats aggregate via bf16 mask matmuls with the variance
    recombination (E[var]+E[mean^2]-mean^2) done post-aggregation; rsqrt via
    the fp32 bit-trick + Newton (no Act table load).
  - queries/residual path ships as bf16 (xq); end-to-end rel err ~1.9e-3
    vs the 2e-2 gate. Measured 256 us (baseline 396/332 us).
"""

import numpy as np

import concourse.bass as bass
import concourse.tile as tile
from concourse import bacc, mybir
from concourse.bass_utils import run_bass_kernel_spmd

C = 256
HW = 4096
NH = 4
D = 64  # head dim
G = 8  # groups
EPS = 1e-5
SCALE = D**-0.5
Q = HW // 2  # queries per core
NJT = HW // 128  # 32 key tiles
NKC = 8  # key chunks (512 keys each) for K/V production
NIC = Q // 512  # 4 query chunks of 512

F32 = mybir.dt.float32
BF16 = mybir.dt.bfloat16
I16 = mybir.dt.int16

# one-op exp on the DVE: exp(t) ~= bitcast_bf16(int16(A16*t + B16)) (Schraudolph
# in bf16 bit space; C=5.5 minimizes rms rel err ~1.8%, +0.5 compensates the
# truncating float->int convert).
A16 = 128.0 / float(np.log(2.0))
B16 = 127.0 * 128.0 - 5.5 + 0.5
LAGP = 4  # AV pairs trail QK/exp by this many jp slots


def _route_pattern(na, nv, n=64):
    """Largest-remainder interleave of n exp tiles across (act, dve)."""
    quota = {"a": na, "v": nv}
    cnt = {"a": 0, "v": 0}
    out = []
    for r in range(n):
        e = max(("a", "v"), key=lambda k: quota[k] * (r + 1) / n - cnt[k])
        cnt[e] += 1
        out.append(e)
    return out


def _hole_pattern(nv):
    """32-tile block routing with tiles 6..11 pinned to Act: the deferred
    normalize finisher lands there in the DVE stream (head-of-line)."""
    slots = list(range(0, 6)) + list(range(12, 32))
    out = ["a"] * 32
    for k in range(nv):
        out[slots[(k * len(slots)) // nv]] = "v"
    return out


def build(finalize=True):
    nc = bacc.Bacc("TRN2", target_bir_lowering=False, debug=False, num_devices=8)

    x = nc.declare_dram_parameter("x", [C, HW], BF16, isOutput=False)
    xq = nc.declare_dram_parameter("xq", [C, Q], BF16, isOutput=False)
    wn2 = nc.declare_dram_parameter("wn2", [128, 2], F32, isOutput=False)
    bn2 = nc.declare_dram_parameter("bn2", [128, 2], F32, isOutput=False)
    wq = nc.declare_dram_parameter("wq", [128, 2, C], BF16, isOutput=False)
    bq2 = nc.declare_dram_parameter("bq2", [128, 2], F32, isOutput=False)
    wk = nc.declare_dram_parameter("wk", [128, 2, C], BF16, isOutput=False)
    bk2 = nc.declare_dram_parameter("bk2", [128, 2], F32, isOutput=False)
    wv = nc.declare_dram_parameter("wv", [128, 2, NH * 65], BF16, isOutput=False)
    vb = nc.declare_dram_parameter("vb", [128, NH * 65], F32, isOutput=False)
    wproj = nc.declare_dram_parameter("wproj", [128, 2, C], BF16, isOutput=False)
    ident = nc.declare_dram_parameter("ident", [128, 128], BF16, isOutput=False)
    bproj2 = nc.declare_dram_parameter("bproj2", [128, 2], F32, isOutput=False)
    gmask = nc.declare_dram_parameter("gmask", [128, 2, 128], BF16, isOutput=False)
    gmaskT = nc.declare_dram_parameter("gmaskT", [128, 2, 128], BF16, isOutput=False)
    out = nc.declare_dram_parameter("out", [C, Q], F32, isOutput=True)

    Exp = mybir.ActivationFunctionType.Exp
    Alu = mybir.AluOpType

    with tile.TileContext(nc) as tc:
        with (
            tc.tile_pool(name="keep", bufs=1) as keep,
            tc.tile_pool(name="consts", bufs=1) as consts,
            tc.tile_pool(name="small", bufs=4) as small,
            tc.tile_pool(name="s_ps", bufs=3, space="PSUM") as s_ps,
            tc.tile_pool(name="acc_ps", bufs=1, space="PSUM") as acc_ps,
            tc.tile_pool(name="exps", bufs=2 * (LAGP + 1)) as expp,
            tc.tile_pool(name="att", bufs=2) as att,
        ):
            # persistent attention operands
            # K per head-pair: rows 0..63 = head 2hp, 64..127 = head 2hp+1
            KZ = [
                keep.tile([128, HW], BF16, tag=f"KZ{t}", name=f"KZ{t}")
                for t in range(2)
            ]
            QT = [
                keep.tile([128, Q], BF16, tag=f"Q{t}", name=f"Q{t}")
                for t in range(2)
            ]
            # V^T with a leading ones column per head: [keys, (head, 1+d)]
            V = keep.tile([128, NJT, NH * 65], BF16)
            XQ = [
                keep.tile([128, Q], BF16, tag=f"XQ{t}", name=f"XQ{t}")
                for t in range(2)
            ]

            with tc.tile_pool(name="xh", bufs=1) as xh:
                X = [
                    xh.tile([128, HW], BF16, tag=f"X{t}", name=f"X{t}")
                    for t in range(2)
                ]
                H = [
                    xh.tile([128, HW], BF16, tag=f"H{t}", name=f"H{t}")
                    for t in range(2)
                ]
                HQ = [
                    xh.tile([128, Q], BF16, tag=f"HQ{t}", name=f"HQ{t}")
                    for t in range(2)
                ]

                # preload the Exp activation table while DMAs run, and wake
                # the gpsimd firmware so its first real op pays no launch cost
                tldum = small.tile([1, 1], F32, tag="tld", name="tld", bufs=1)
                nc.vector.memset(tldum, 1.0)
                nc.scalar.activation(out=tldum, in_=tldum, func=Exp)
                gpdum = small.tile([1, 1], F32, tag="gpd", name="gpd", bufs=1)
                nc.gpsimd.memset(gpdum, 0.0)

                # ---- x DMA in chunks, bn_stats per chunk ----
                st = [
                    small.tile([128, 8, 6], F32, tag=f"bnst{t}", name=f"bnst{t}")
                    for t in range(2)
                ]
                JW = small.tile([128, 128], BF16, tag="junkw", name="junkw", bufs=1)
                JR = small.tile([128, 512], BF16, tag="junkr", name="junkr", bufs=1)
                nc.vector.memset(JW, 0.0)
                for ch in range(4):
                    for t in range(2):
                        eng = nc.sync if t == 0 else nc.scalar
                        eng.dma_start(
                            out=X[t][:, ch * 1024 : (ch + 1) * 1024],
                            in_=x[t * 128 : (t + 1) * 128, ch * 1024 : (ch + 1) * 1024],
                        )
                        xr = X[t].rearrange("p (n f) -> p n f", f=512)
                        for s in (2 * ch, 2 * ch + 1):
                            nc.vector.bn_stats(out=st[t][:, s], in_=xr[:, s])
                    if ch == 3:
                        # memset lands here in the DVE stream: the PE warmup
                        # below starts as stats wind down, so the array is
                        # still at full clock when the real matmuls arrive
                        nc.vector.memset(JR, 0.0)
                wps = s_ps.tile([128, 1024], F32, tag="sps", name="wps")
                for w in range(6):
                    nc.tensor.matmul(
                        out=wps[:, 0:512], lhsT=JW, rhs=JR, start=True, stop=True
                    )
                for t in range(2):
                    eng = nc.sync if t == 0 else nc.scalar
                    eng.dma_start(out=XQ[t], in_=xq[t * 128 : (t + 1) * 128, :])

                # ---- weights (after the x chunks in queue order) ----
                GM = consts.tile([128, 2, 128], BF16)
                nc.sync.dma_start(out=GM, in_=gmask[:])
                GMT = consts.tile([128, 2, 128], BF16)
                nc.scalar.dma_start(out=GMT, in_=gmaskT[:])
                WN = consts.tile([128, 2], F32)
                nc.sync.dma_start(out=WN, in_=wn2[:])
                BN = consts.tile([128, 2], F32)
                nc.scalar.dma_start(out=BN, in_=bn2[:])
                WQ = consts.tile([128, 2, C], BF16)
                nc.sync.dma_start(out=WQ, in_=wq[:])
                BQ = consts.tile([128, 2], F32)
                nc.scalar.dma_start(out=BQ, in_=bq2[:])
                WK = consts.tile([128, 2, C], BF16)
                nc.sync.dma_start(out=WK, in_=wk[:])
                BK = consts.tile([128, 2], F32)
                nc.scalar.dma_start(out=BK, in_=bk2[:])
                WV = consts.tile([128, 2, NH * 65], BF16)
                nc.sync.dma_start(out=WV, in_=wv[:])
                VB = consts.tile([128, NH * 65], F32)
                nc.scalar.dma_start(out=VB, in_=vb[:])
                WP = consts.tile([128, 2, C], BF16)
                nc.sync.dma_start(out=WP, in_=wproj[:])
                IDENT = consts.tile([128, 128], BF16)
                nc.scalar.dma_start(out=IDENT, in_=ident[:])
                BP = consts.tile([128, 2], F32)
                nc.scalar.dma_start(out=BP, in_=bproj2[:])

                # ---- GroupNorm statistics ----
                # bf16 stats operands keep the aggregation matmuls off the
                # slow fp32 PE path; precision impact ~0.2% on rstd.
                mv3 = small.tile([128, 2, 3], BF16)  # [:, t, (mean, var, mean^2)]
                for t in range(2):
                    mv = small.tile([128, 2], F32, tag="bnmv")
                    nc.vector.bn_aggr(out=mv, in_=st[t])
                    nc.vector.tensor_copy(out=mv3[:, t, 0:2], in_=mv)
                    nc.vector.tensor_tensor(
                        out=mv3[:, t, 2:3], in0=mv[:, 0:1], in1=mv[:, 0:1],
                        op=Alu.mult,
                    )

                gps = s_ps.tile([128, 1024], F32, tag="sps", name="gnps")
                for t in range(2):
                    nc.tensor.matmul(
                        out=gps[:, 0:3], lhsT=GM[:, t], rhs=mv3[:, t],
                        start=(t == 0), stop=(t == 1),
                    )
                gsb = small.tile([128, 3], F32)
                nc.vector.tensor_copy(out=gsb, in_=gps[:, 0:3])
                # gstat rows 0..8: col0 = group mean, col1 = rsqrt(var);
                # rows 8..128 stay zero for the padded broadcast matmul.
                gstat = small.tile([128, 2], BF16)
                nc.vector.memset(gstat, 0.0)
                nc.vector.tensor_copy(out=gstat[:G, 0:1], in_=gsb[:G, 0:1])
                # var_g = E[var] + E[mean^2] - mean_g^2 (EPS=1e-5 negligible
                # at var ~ 1)
                gvar = small.tile([G, 1], F32)
                nc.vector.tensor_tensor(
                    out=gvar, in0=gsb[:G, 1:2], in1=gsb[:G, 2:3], op=Alu.add
                )
                gm2 = small.tile([G, 1], F32, tag="gm2", name="gm2")
                nc.vector.tensor_tensor(
                    out=gm2, in0=gsb[:G, 0:1], in1=gsb[:G, 0:1], op=Alu.mult
                )
                nc.vector.tensor_tensor(
                    out=gvar, in0=gvar, in1=gm2, op=Alu.subtract
                )
                # rsqrt via the fp32 bit-trick seed + Newton step (keeps the
                # Act engine free of Ln table loads)
                gvi = gvar.bitcast(mybir.dt.int32)
                y0i = small.tile([G, 1], mybir.dt.int32, tag="y0i", name="y0i")
                nc.vector.tensor_scalar(
                    out=y0i, in0=gvi, scalar1=1, scalar2=0,
                    op0=Alu.logical_shift_right, op1=Alu.bitwise_or,
                )
                nc.vector.tensor_scalar(
                    out=y0i, in0=y0i, scalar1=-1, scalar2=0x5F3759DF,
                    op0=Alu.mult, op1=Alu.add,
                )
                y = y0i.bitcast(F32)
                yt = small.tile([G, 1], F32, tag="yt", name="yt")
                for _ in range(1):
                    nc.vector.tensor_tensor(out=yt, in0=y, in1=y, op=Alu.mult)
                    nc.vector.tensor_tensor(out=yt, in0=yt, in1=gvar, op=Alu.mult)
                    nc.vector.tensor_scalar(
                        out=yt, in0=yt, scalar1=-0.5, scalar2=1.5,
                        op0=Alu.mult, op1=Alu.add,
                    )
                    nc.vector.tensor_tensor(out=y, in0=y, in1=yt, op=Alu.mult)
                nc.vector.tensor_copy(out=gstat[:G, 1:2], in_=y)

                # broadcast group stats back to channels
                AB = []  # [t] -> [128, 2] (alpha, beta)
                for t in range(2):
                    bc = s_ps.tile([128, 1024], F32, tag="sps", name="bcst")
                    nc.tensor.matmul(out=bc[:, 0:2], lhsT=GMT[:, t], rhs=gstat)
                    bsb = small.tile([128, 2], F32, tag="bsb", name="bsb")
                    nc.vector.tensor_copy(out=bsb, in_=bc[:, 0:2])
                    ab = small.tile([128, 2], F32, tag=f"ab{t}", name=f"ab{t}")
                    # alpha = rstd * w
                    nc.vector.tensor_tensor(
                        out=ab[:, 0:1], in0=bsb[:, 1:2], in1=WN[:, t : t + 1],
                        op=Alu.mult,
                    )
                    # beta = b - mean * alpha
                    nc.vector.tensor_tensor(
                        out=ab[:, 1:2], in0=bsb[:, 0:1], in1=ab[:, 0:1],
                        op=Alu.mult,
                    )
                    nc.vector.tensor_tensor(
                        out=ab[:, 1:2], in0=BN[:, t : t + 1], in1=ab[:, 1:2],
                        op=Alu.subtract,
                    )
                    AB.append(ab)

                # ---- chunked production helpers ----
                def hq_chunk(c, eng=None):  # normalized queries, 512 cols
                    for t in range(2):
                        (eng or nc.gpsimd).tensor_scalar(
                            out=HQ[t][:, c * 512 : (c + 1) * 512],
                            in0=XQ[t][:, c * 512 : (c + 1) * 512],
                            scalar1=AB[t][:, 0:1], scalar2=AB[t][:, 1:2],
                            op0=Alu.mult, op1=Alu.add,
                        )

                def h_chunk(c):  # normalized keys, 512 cols (gpsimd: pure
                    # SBUF->SBUF, keeps the DVE free for psum drains)
                    for t in range(2):
                        nc.gpsimd.tensor_scalar(
                            out=H[t][:, c * 512 : (c + 1) * 512],
                            in0=X[t][:, c * 512 : (c + 1) * 512],
                            scalar1=AB[t][:, 0:1], scalar2=AB[t][:, 1:2],
                            op0=Alu.mult, op1=Alu.add,
                        )

                def q_chunk(c):  # q projection for queries 512c.. (both t)
                    for t in range(2):
                        ps = s_ps.tile([128, 1024], F32, tag="sps", name="qps")
                        for ct in range(2):
                            nc.tensor.matmul(
                                out=ps[:, 0:512],
                                lhsT=WQ[:, ct, t * 128 : (t + 1) * 128],
                                rhs=HQ[ct][:, c * 512 : (c + 1) * 512],
                                start=(ct == 0), stop=(ct == 1),
                            )
                        nc.vector.tensor_scalar_add(
                            out=QT[t][:, c * 512 : (c + 1) * 512],
                            in0=ps[:, 0:512], scalar1=BQ[:, t : t + 1],
                        )

                def k_piece(n, hp):  # K head-pair hp for keys 512n..
                    ps = s_ps.tile([128, 1024], F32, tag="sps", name="kps")
                    for ct in range(2):
                        nc.tensor.matmul(
                            out=ps[:, 0:512],
                            lhsT=WK[:, ct, hp * 128 : (hp + 1) * 128],
                            rhs=H[ct][:, n * 512 : (n + 1) * 512],
                            start=(ct == 0), stop=(ct == 1),
                        )
                    nc.vector.tensor_scalar_add(
                        out=KZ[hp][:, n * 512 : (n + 1) * 512],
                        in0=ps[:, 0:512], scalar1=BK[:, hp : hp + 1],
                    )

                def v_piece(n, half):  # V^T for key tiles 4n+2*half(+1)
                    j0 = 4 * n + 2 * half
                    ps = s_ps.tile([128, 1024], F32, tag="sps", name="vps")
                    for jo in range(2):
                        for ct in range(2):
                            nc.tensor.matmul(
                                out=ps[:, jo * 512 : jo * 512 + NH * 65],
                                lhsT=H[ct][:, (j0 + jo) * 128 : (j0 + jo + 1) * 128],
                                rhs=WV[:, ct],
                                start=(ct == 0), stop=(ct == 1),
                            )
                        nc.vector.tensor_tensor(
                            out=V[:, j0 + jo],
                            in0=ps[:, jo * 512 : jo * 512 + NH * 65],
                            in1=VB, op=Alu.add,
                        )

                # minimal chain to the first QK: chunk 0 of HQ/H/Q/K.
                # h(0) on gpsimd and hq(0) on the DVE run in parallel.
                h_chunk(0)
                hq_chunk(0, nc.vector)
                k_piece(0, 0)
                q_chunk(0)

                # ic0 production schedule: jp slot -> tasks, per hp.
                # V chunk m lands at slot 2m (just in time for its own AVs),
                # K chunk m+1 and H chunk m+2 at slot 2m+1; hp1 only needs
                # its own K head-pair. q/hq chunks ride along for later ics.
                prod0, prod1 = {}, {}
                prod0[0] = [
                    lambda: h_chunk(1),
                    lambda: v_piece(0, 0), lambda: v_piece(0, 1),
                ]
                for m in range(1, NKC):
                    tasks = []
                    if m + 1 < NKC:
                        tasks.append(lambda c=m + 1: h_chunk(c))
                    tasks.append(lambda c=m: k_piece(c, 0))
                    prod0[2 * m - 1] = tasks
                    prod0[2 * m] = [
                        lambda c=m: v_piece(c, 0), lambda c=m: v_piece(c, 1)
                    ]
                for qi, p in ((1, 4), (2, 8), (3, 12)):
                    prod0[p] = prod0.get(p, []) + [
                        lambda c=qi: hq_chunk(c), lambda c=qi: q_chunk(c)
                    ]
                prod0[14] = prod0.get(14, []) + [lambda: k_piece(0, 1)]
                for m in range(1, NKC):
                    prod1[2 * m - 1] = [lambda c=m: k_piece(c, 1)]

                # exp tile routing across THREE consumers: Act (native Exp),
                # DVE (one-op Schraudolph from psum), gpsimd (Schraudolph from
                # a DMA-staged SBUF copy -- gpsimd cannot read psum; the copy
                # rides the idle sync DMA queue). ic0 leans on Act (DVE does
                # production drains there, gpsimd the h chunks).
                exp_cnt = [0]
                PAT0 = _route_pattern(48, 16)
                PAT = _route_pattern(37, 27)

                def do_exp(S, E):
                    i = exp_cnt[0]
                    exp_cnt[0] += 1
                    lab = (PAT0 if i < 64 else PAT)[i % 64]
                    if lab == "v":
                        nc.vector.tensor_scalar(
                            out=E, in0=S, scalar1=A16 * SCALE, scalar2=B16,
                            op0=Alu.mult, op1=Alu.add,
                        )
                    else:
                        nc.scalar.activation(
                            out=E.bitcast(BF16), in_=S, func=Exp, scale=SCALE
                        )

                # ---- attention + projection (oT form: queries on psum
                # partitions, exp(scores) streamed as the stationary operand,
                # per-partition softmax normalization).
                # AV matmuls and per-block finishers (normalize, transpose,
                # proj, residual) are DEFERRED through a global work queue so
                # the next block's QK/exp stream is emitted ahead of them --
                # the in-order engines never serialize at hp/ic boundaries.
                pend = []      # (emit_av_closure, block_key)
                finishers = {}  # block_key -> closure run after its last AV

                def pump(k):
                    for _ in range(k):
                        if not pend:
                            break
                        fn, key = pend.pop(0)
                        fn()
                        if key in finishers and not any(
                            k2 == key for _, k2 in pend
                        ):
                            finishers.pop(key)()

                for ic in range(NIC):
                    oTn = att.tile([128, 4, 256], BF16, tag="oTn", name="oTn")
                    OSB = att.tile([128, 2, 512], BF16, tag="osb", name="osb")
                    for hp in range(2):
                        oT = [
                            acc_ps.tile(
                                [128, 4, 68], F32, tag=f"ot{h2}", name=f"ot{h2}"
                            )
                            for h2 in range(2)
                        ]

                        def qk_into(S, j, hp=hp, ic=ic):
                            for h2 in range(2):
                                nc.tensor.matmul(
                                    out=S[:, h2 * 512 : (h2 + 1) * 512],
                                    lhsT=KZ[hp][
                                        h2 * 64 : (h2 + 1) * 64,
                                        j * 128 : (j + 1) * 128,
                                    ],
                                    rhs=QT[hp][
                                        h2 * 64 : (h2 + 1) * 64,
                                        ic * 512 : (ic + 1) * 512,
                                    ],
                                    start=True, stop=True,
                                )

                        def av_from(E, j, oT=oT, hp=hp):
                            for h2 in range(2):
                                head = 2 * hp + h2
                                for isub in range(4):
                                    nc.tensor.matmul(
                                        out=oT[h2][:, isub, 0:65],
                                        lhsT=E[
                                            :,
                                            h2 * 512 + isub * 128 : h2 * 512
                                            + (isub + 1) * 128,
                                        ].bitcast(BF16),
                                        rhs=V[:, j, head * 65 : (head + 1) * 65],
                                        start=(j == 0 and isub == 0),
                                        stop=(j == NJT - 1 and isub == 3),
                                    )

                        for jp in range(NJT // 2):
                            if ic == 0:
                                sched = prod0 if hp == 0 else prod1
                                for task in sched.get(jp, ()):
                                    task()
                            for jo in range(2):
                                j = 2 * jp + jo
                                S = s_ps.tile(
                                    [128, 1024], F32, tag="sps", name="s"
                                )
                                qk_into(S, j)
                                E = expp.tile(
                                    [128, 1024], I16, tag="exps", name="e"
                                )
                                do_exp(S, E)
                                pend.append(
                                    (lambda E=E, j=j, f=av_from: f(E, j), (ic, hp))
                                )
                            if len(pend) > 2 * LAGP:
                                pump(2)

                        def mk_norm(oT=oT, hp=hp, oTn=oTn):
                            def fin():
                                # normalize by the ones-column sums (per-
                                # partition; one strided reciprocal covers all
                                # 4 sub-tiles)
                                for h2 in range(2):
                                    head = 2 * hp + h2
                                    r4 = small.tile(
                                        [128, 4], F32, tag="recip", name="recip"
                                    )
                                    nc.vector.reciprocal(
                                        out=r4,
                                        in_=oT[h2][:, :, 64:65].rearrange(
                                            "p a b -> p (a b)"
                                        ),
                                    )
                                    for isub in range(4):
                                        nc.vector.tensor_scalar_mul(
                                            out=oTn[
                                                :, isub, head * 64 : (head + 1) * 64
                                            ],
                                            in0=oT[h2][:, isub, 0:64],
                                            scalar1=r4[:, isub : isub + 1],
                                        )
                            return fin

                        finishers[(ic, hp)] = mk_norm()

                    def mk_boundary(norm1=finishers[(ic, 1)], oTn=oTn, OSB=OSB, ic=ic):
                        def fin():
                            norm1()
                            # transpose oTn -> [channels, 512 queries]: all 8
                            # land in one score-pool psum bank (chained group)
                            tps = s_ps.tile([128, 1024], F32, tag="sps", name="tps")
                            tpb = tps.bitcast(BF16)  # [128, 2048]
                            for ct in range(2):
                                for isub in range(4):
                                    k8 = ct * 4 + isub
                                    nc.tensor.matmul(
                                        tpb[:, k8 * 128 : (k8 + 1) * 128],
                                        oTn[:, isub, ct * 128 : (ct + 1) * 128],
                                        IDENT,
                                        is_transpose=True,
                                        start=(k8 == 0), stop=(k8 == 7),
                                    )
                            for ct in range(2):
                                nc.vector.tensor_copy(
                                    out=OSB[:, ct],
                                    in_=tpb[:, ct * 512 : (ct + 1) * 512],
                                )
                            # proj + bias + residual
                            pj = s_ps.tile([128, 1024], F32, tag="sps", name="pj")
                            for mt in range(2):
                                for ct in range(2):
                                    nc.tensor.matmul(
                                        out=pj[:, mt * 512 : (mt + 1) * 512],
                                        lhsT=WP[:, ct, mt * 128 : (mt + 1) * 128],
                                        rhs=OSB[:, ct],
                                        start=(ct == 0), stop=(ct == 1),
                                    )
                                ob = att.tile(
                                    [128, 512], F32, tag="outsb", name="outsb"
                                )
                                nc.vector.scalar_tensor_tensor(
                                    out=ob, in0=pj[:, mt * 512 : (mt + 1) * 512],
                                    scalar=BP[:, mt : mt + 1],
                                    in1=XQ[mt][:, ic * 512 : (ic + 1) * 512],
                                    op0=Alu.add, op1=Alu.add,
                                )
                                nc.sync.dma_start(
                                    out=out[
                                        mt * 128 : (mt + 1) * 128,
                                        ic * 512 : (ic + 1) * 512,
                                    ],
                                    in_=ob,
                                )
                        return fin

                    finishers[(ic, 1)] = mk_boundary()
                pump(len(pend))
    if finalize:
        nc.finalize()
    return nc


def _prep_weights(norm_w, norm_b, qkv_w, qkv_b, proj_w, proj_b):
    """Host-side layout (pure reshapes/transposes + dtype casts of weights)."""
    import ml_dtypes

    f = np.float32
    cdt = ml_dtypes.bfloat16

    def ctile(v):  # (256,) -> (128, 2) per channel-tile columns
        return np.ascontiguousarray(np.asarray(v).reshape(2, 128).T, dtype=f)

    def ptile(m):  # (256, N) -> (128, 2, N)
        return np.ascontiguousarray(
            np.asarray(m).reshape(2, 128, -1).transpose(1, 0, 2), dtype=f
        )

    qkv_w = np.asarray(qkv_w)
    qkv_b = np.asarray(qkv_b)
    wqT = qkv_w[:C].T  # (256, 256)
    wkT = qkv_w[C : 2 * C].T  # (256, 256): out col o = head-pair*128 + row
    wvm = qkv_w[2 * C :]  # (256, 256)
    wvT = np.zeros((C, NH * 65), dtype=f)
    vb = np.zeros((128, NH * 65), dtype=f)
    for h in range(NH):
        wvT[:, h * 65 : h * 65 + 64] = wvm[h * 64 : (h + 1) * 64].T
        vb[:, h * 65 : h * 65 + 64] = qkv_b[
            2 * C + h * 64 : 2 * C + (h + 1) * 64
        ][None, :]
        vb[:, h * 65 + 64] = 1.0  # ones column -> denominator at oT column 64
    # zero-padded group masks (value 1/32 for group-mean aggregation; one-hot
    # transpose for the broadcast back to channels)
    gm = np.zeros((C, 128), dtype=f)
    for c in range(C):
        gm[c, c // 32] = 1.0 / 32.0
    gmaskT = np.zeros((128, 2, 128), dtype=f)
    for c in range(C):
        gmaskT[c // 32, c // 128, c % 128] = 1.0

    return dict(
        wn2=ctile(norm_w),
        bn2=ctile(norm_b),
        wq=ptile(wqT).astype(cdt),
        bq2=ctile(qkv_b[:C]),
        wk=ptile(wkT).astype(cdt),
        bk2=ctile(qkv_b[C : 2 * C]),
        wv=ptile(wvT).astype(cdt),
        vb=vb,
        wproj=ptile(np.asarray(proj_w).T).astype(cdt),
        ident=np.eye(128, dtype=cdt),
        bproj2=ctile(proj_b),
        gmask=ptile(gm).astype(cdt),
        gmaskT=gmaskT.astype(cdt),
    )


_NC_CACHE = {}
_RUN_OPTS = {}  # extra kwargs for run_bass_kernel_spmd (test harness sets trace)
LAST_RESULT = None


def _get_nc():
    if "nc" not in _NC_CACHE:
        _NC_CACHE["nc"] = build()
    return _NC_CACHE["nc"]


def kernel(x, norm_w, norm_b, qkv_w, qkv_b, proj_w, proj_b, **_):
    import ml_dtypes

    nc = _get_nc()
    w = _prep_weights(norm_w, norm_b, qkv_w, qkv_b, proj_w, proj_b)
    x = np.asarray(x, dtype=np.float32)
    Bv, Cv, Hv, Wv = x.shape
    xf = x.reshape(Bv, Cv, Hv * Wv)
    xb = xf.astype(ml_dtypes.bfloat16)
    in_maps = []
    for j in range(8):
        b, qh = j // 2, j % 2
        m = dict(w)
        m["x"] = np.ascontiguousarray(xb[b])
        m["xq"] = np.ascontiguousarray(xb[b][:, qh * Q : (qh + 1) * Q])
        in_maps.append(m)
    res = run_bass_kernel_spmd(nc, in_maps, core_ids=list(range(8)), **_RUN_OPTS)
    global LAST_RESULT
    LAST_RESULT = res
    outf = np.empty((Bv, Cv, Hv * Wv), dtype=np.float32)
    for j in range(8):
        b, qh = j // 2, j % 2
        outf[b][:, qh * Q : (qh + 1) * Q] = res.results[j]["out"]
    return outf.reshape(Bv, Cv, Hv, Wv)


# revision 40
# speedup vs baseline: 1.2340x; 1.0132x over previous
"""AttentionBlock (GroupNorm -> qkv 1x1 -> 4-head attention over 4096 tokens
-> proj 1x1 -> residual) distributed over 8 TRN2 NeuronCores.

Sharding: zero-communication query sharding. Core j handles batch b = j//2 and
query half qh = j%2 (2048 of the 4096 spatial positions). Each core loads the
full x[b] (256, 4096), computes GroupNorm + K/V over all keys, Q only for its
2048 queries, and writes its (256, 2048) output slice.

Structure (bf16 PE shapes -- measured on HW: fp8/DoubleRow gives no PE gain;
512-col bf16 matmuls and 65-col AV matmuls with hidden weight loads are the
throughput-optimal shapes; 64-partition matmul pairs at row quadrants 0/64
execute concurrently on the PE):
  - scores transposed ([keys, queries], lhsT=k rhs=q) so exp output feeds AV
    directly; denominator rides as a ones column in V^T; per-partition
    reciprocal normalization (no cross-partition broadcasts).
  - K stored per head-PAIR [2x64ch, keys] (real 64-deep contract, base
    partition 0/64): halves K production vs zero-padded per-head tiles, and
    the two heads' QK matmuls run in parallel on disjoint PE row-quadrants.
  - exp tiles split ~60/40 between the Act engine (native Exp) and the DVE
    (one tensor_scalar: bits = A16*scale*s + B16 -> int16 = bf16 pattern of
    e^s, Schraudolph; rel-err ~2%, damped ~30x by the residual). Only these
    two engines can read PSUM, so they bound softmax throughput.
  - 3 score buffers (6 psum banks) + 2 oT accumulators (2 banks); transposes,
    proj and K/V/Q production psums time-share the score pool. AV pairs trail
    QK/exp by LAGP slots so the in-order PE stream never waits on a fresh exp.
  - GroupNorm stats aggregate via bf16 mask matmuls with the variance
    recombination (E[var]+E[mean^2]-mean^2) done post-aggregation; rsqrt via
    the fp32 bit-trick + Newton (no Act table load).
  - queries/residual path ships as bf16 (xq); end-to-end rel err ~1.9e-3
    vs the 2e-2 gate. Measured 256 us (baseline 396/332 us).
"""

import numpy as np

import concourse.bass as bass
import concourse.tile as tile
from concourse import bacc, mybir
from concourse.bass_utils import run_bass_kernel_spmd

C = 256
HW = 4096
NH = 4
D = 64  # head dim
G = 8  # groups
EPS = 1e-5
SCALE = D**-0.5
Q = HW // 2  # queries per core
NJT = HW // 128  # 32 key tiles
NKC = 8  # key chunks (512 keys each) for K/V production
NIC = Q // 512  # 4 query chunks of 512

F32 = mybir.dt.float32
BF16 = mybir.dt.bfloat16
I16 = mybir.dt.int16

# one-op exp on the DVE: exp(t) ~= bitcast_bf16(int16(A16*t + B16)) (Schraudolph
# in bf16 bit space; C=5.5 minimizes rms rel err ~1.8%, +0.5 compensates the
# truncating float->int convert).
A16 = 128.0 / float(np.log(2.0))
B16 = 127.0 * 128.0 - 5.5 + 0.5
LAGP = 4  # AV pairs trail QK/exp by this many jp slots


def _route_pattern(na, nv, n=64):
    """Largest-remainder interleave of n exp tiles across (act, dve)."""
    quota = {"a": na, "v": nv}
    cnt = {"a": 0, "v": 0}
    out = []
    for r in range(n):
        e = max(("a", "v"), key=lambda k: quota[k] * (r + 1) / n - cnt[k])
        cnt[e] += 1
        out.append(e)
    return out


def _hole_pattern(nv):
    """32-tile block routing with tiles 6..11 pinned to Act: the deferred
    normalize finisher lands there in the DVE stream (head-of-line)."""
    slots = list(range(0, 6)) + list(range(12, 32))
    out = ["a"] * 32
    for k in range(nv):
        out[slots[(k * len(slots)) // nv]] = "v"
    return out


def build(finalize=True):
    nc = bacc.Bacc("TRN2", target_bir_lowering=False, debug=False, num_devices=8)

    x = nc.declare_dram_parameter("x", [C, HW], BF16, isOutput=False)
    xq = nc.declare_dram_parameter("xq", [C, Q], BF16, isOutput=False)
    wn2 = nc.declare_dram_parameter("wn2", [128, 2], F32, isOutput=False)
    bn2 = nc.declare_dram_parameter("bn2", [128, 2], F32, isOutput=False)
    wq = nc.declare_dram_parameter("wq", [128, 2, C], BF16, isOutput=False)
    bq2 = nc.declare_dram_parameter("bq2", [128, 2], F32, isOutput=False)
    wk = nc.declare_dram_parameter("wk", [128, 2, C], BF16, isOutput=False)
    bk2 = nc.declare_dram_parameter("bk2", [128, 2], F32, isOutput=False)
    wv = nc.declare_dram_parameter("wv", [128, 2, NH * 65], BF16, isOutput=False)
    vb = nc.declare_dram_parameter("vb", [128, NH * 65], F32, isOutput=False)
    wproj = nc.declare_dram_parameter("wproj", [128, 2, C], BF16, isOutput=False)
    ident = nc.declare_dram_parameter("ident", [128, 128], BF16, isOutput=False)
    bproj2 = nc.declare_dram_parameter("bproj2", [128, 2], F32, isOutput=False)
    gmask = nc.declare_dram_parameter("gmask", [128, 2, 128], BF16, isOutput=False)
    gmaskT = nc.declare_dram_parameter("gmaskT", [128, 2, 128], BF16, isOutput=False)
    out = nc.declare_dram_parameter("out", [C, Q], F32, isOutput=True)

    Exp = mybir.ActivationFunctionType.Exp
    Alu = mybir.AluOpType

    with tile.TileContext(nc) as tc:
        with (
            tc.tile_pool(name="keep", bufs=1) as keep,
            tc.tile_pool(name="consts", bufs=1) as consts,
            tc.tile_pool(name="small", bufs=4) as small,
            tc.tile_pool(name="s_ps", bufs=3, space="PSUM") as s_ps,
            tc.tile_pool(name="acc_ps", bufs=1, space="PSUM") as acc_ps,
            tc.tile_pool(name="exps", bufs=2 * (LAGP + 1)) as expp,
            tc.tile_pool(name="att", bufs=2) as att,
        ):
            # persistent attention operands
            # K per head-pair: rows 0..63 = head 2hp, 64..127 = head 2hp+1
            KZ = [
                keep.tile([128, HW], BF16, tag=f"KZ{t}", name=f"KZ{t}")
                for t in range(2)
            ]
            QT = [
                keep.tile([128, Q], BF16, tag=f"Q{t}", name=f"Q{t}")
                for t in range(2)
            ]
            # V^T with a leading ones column per head: [keys, (head, 1+d)]
            V = keep.tile([128, NJT, NH * 65], BF16)
            XQ = [
                keep.tile([128, Q], BF16, tag=f"XQ{t}", name=f"XQ{t}")
                for t in range(2)
            ]

            with tc.tile_pool(name="xh", bufs=1) as xh:
                X = [
                    xh.tile([128, HW], BF16, tag=f"X{t}", name=f"X{t}")
                    for t in range(2)
                ]
                H = [
                    xh.tile([128, HW], BF16, tag=f"H{t}", name=f"H{t}")
                    for t in range(2)
                ]
                HQ = [
                    xh.tile([128, Q], BF16, tag=f"HQ{t}", name=f"HQ{t}")
                    for t in range(2)
                ]

                # preload the Exp activation table while DMAs run, and wake
                # the gpsimd firmware so its first real op pays no launch cost
                tldum = small.tile([1, 1], F32, tag="tld", name="tld", bufs=1)
                nc.vector.memset(tldum, 1.0)
                nc.scalar.activation(out=tldum, in_=tldum, func=Exp)
                gpdum = small.tile([1, 1], F32, tag="gpd", name="gpd", bufs=1)
                nc.gpsimd.memset(gpdum, 0.0)

                # ---- x DMA in chunks, bn_stats per chunk ----
                st = [
                    small.tile([128, 8, 6], F32, tag=f"bnst{t}", name=f"bnst{t}")
                    for t in range(2)
                ]
                JW = small.tile([128, 128], BF16, tag="junkw", name="junkw", bufs=1)
                JR = small.tile([128, 512], BF16, tag="junkr", name="junkr", bufs=1)
                nc.vector.memset(JW, 0.0)
                for ch in range(4):
                    for t in range(2):
                        eng = nc.sync if t == 0 else nc.scalar
                        eng.dma_start(
                            out=X[t][:, ch * 1024 : (ch + 1) * 1024],
                            in_=x[t * 128 : (t + 1) * 128, ch * 1024 : (ch + 1) * 1024],
                        )
                        xr = X[t].rearrange("p (n f) -> p n f", f=512)
                        for s in (2 * ch, 2 * ch + 1):
                            nc.vector.bn_stats(out=st[t][:, s], in_=xr[:, s])
                    if ch == 3:
                        # memset lands here in the DVE stream: the PE warmup
                        # below starts as stats wind down, so the array is
                        # still at full clock when the real matmuls arrive
                        nc.vector.memset(JR, 0.0)
                wps = s_ps.tile([128, 1024], F32, tag="sps", name="wps")
                for w in range(6):
                    nc.tensor.matmul(
                        out=wps[:, 0:512], lhsT=JW, rhs=JR, start=True, stop=True
                    )
                # ---- weights (small GroupNorm masks FIRST: the stats
                # matmul needs them right after bn_aggr; xq can wait) ----
                GM = consts.tile([128, 2, 128], BF16)
                nc.sync.dma_start(out=GM, in_=gmask[:])
                GMT = consts.tile([128, 2, 128], BF16)
                nc.scalar.dma_start(out=GMT, in_=gmaskT[:])
                WN = consts.tile([128, 2], F32)
                nc.sync.dma_start(out=WN, in_=wn2[:])
                BN = consts.tile([128, 2], F32)
                nc.scalar.dma_start(out=BN, in_=bn2[:])
                for t in range(2):
                    eng = nc.sync if t == 0 else nc.scalar
                    eng.dma_start(out=XQ[t], in_=xq[t * 128 : (t + 1) * 128, :])
                WQ = consts.tile([128, 2, C], BF16)
                nc.sync.dma_start(out=WQ, in_=wq[:])
                BQ = consts.tile([128, 2], F32)
                nc.scalar.dma_start(out=BQ, in_=bq2[:])
                WK = consts.tile([128, 2, C], BF16)
                nc.sync.dma_start(out=WK, in_=wk[:])
                BK = consts.tile([128, 2], F32)
                nc.scalar.dma_start(out=BK, in_=bk2[:])
                WV = consts.tile([128, 2, NH * 65], BF16)
                nc.sync.dma_start(out=WV, in_=wv[:])
                VB = consts.tile([128, NH * 65], F32)
                nc.scalar.dma_start(out=VB, in_=vb[:])
                WP = consts.tile([128, 2, C], BF16)
                nc.sync.dma_start(out=WP, in_=wproj[:])
                IDENT = consts.tile([128, 128], BF16)
                nc.scalar.dma_start(out=IDENT, in_=ident[:])
                BP = consts.tile([128, 2], F32)
                nc.scalar.dma_start(out=BP, in_=bproj2[:])

                # ---- GroupNorm statistics ----
                # bf16 stats operands keep the aggregation matmuls off the
                # slow fp32 PE path; precision impact ~0.2% on rstd.
                mv3 = small.tile([128, 2, 3], BF16)  # [:, t, (mean, var, mean^2)]
                for t in range(2):
                    mv = small.tile([128, 2], F32, tag="bnmv")
                    nc.vector.bn_aggr(out=mv, in_=st[t])
                    nc.vector.tensor_copy(out=mv3[:, t, 0:2], in_=mv)
                    nc.vector.tensor_tensor(
                        out=mv3[:, t, 2:3], in0=mv[:, 0:1], in1=mv[:, 0:1],
                        op=Alu.mult,
                    )

                gps = s_ps.tile([128, 1024], F32, tag="sps", name="gnps")
                for t in range(2):
                    nc.tensor.matmul(
                        out=gps[:, 0:3], lhsT=GM[:, t], rhs=mv3[:, t],
                        start=(t == 0), stop=(t == 1),
                    )
                gsb = small.tile([128, 3], F32)
                nc.vector.tensor_copy(out=gsb, in_=gps[:, 0:3])
                # gstat rows 0..8: col0 = group mean, col1 = rsqrt(var);
                # rows 8..128 stay zero for the padded broadcast matmul.
                gstat = small.tile([128, 2], BF16)
                nc.vector.memset(gstat, 0.0)
                nc.vector.tensor_copy(out=gstat[:G, 0:1], in_=gsb[:G, 0:1])
                # var_g = E[var] + E[mean^2] - mean_g^2 (EPS=1e-5 negligible
                # at var ~ 1)
                gvar = small.tile([G, 1], F32)
                nc.vector.tensor_tensor(
                    out=gvar, in0=gsb[:G, 1:2], in1=gsb[:G, 2:3], op=Alu.add
                )
                gm2 = small.tile([G, 1], F32, tag="gm2", name="gm2")
                nc.vector.tensor_tensor(
                    out=gm2, in0=gsb[:G, 0:1], in1=gsb[:G, 0:1], op=Alu.mult
                )
                nc.vector.tensor_tensor(
                    out=gvar, in0=gvar, in1=gm2, op=Alu.subtract
                )
                # rsqrt via the fp32 bit-trick seed + Newton step (keeps the
                # Act engine free of Ln table loads)
                gvi = gvar.bitcast(mybir.dt.int32)
                y0i = small.tile([G, 1], mybir.dt.int32, tag="y0i", name="y0i")
                nc.vector.tensor_scalar(
                    out=y0i, in0=gvi, scalar1=1, scalar2=0,
                    op0=Alu.logical_shift_right, op1=Alu.bitwise_or,
                )
                nc.vector.tensor_scalar(
                    out=y0i, in0=y0i, scalar1=-1, scalar2=0x5F3759DF,
                    op0=Alu.mult, op1=Alu.add,
                )
                y = y0i.bitcast(F32)
                yt = small.tile([G, 1], F32, tag="yt", name="yt")
                for _ in range(1):
                    nc.vector.tensor_tensor(out=yt, in0=y, in1=y, op=Alu.mult)
                    nc.vector.tensor_tensor(out=yt, in0=yt, in1=gvar, op=Alu.mult)
                    nc.vector.tensor_scalar(
                        out=yt, in0=yt, scalar1=-0.5, scalar2=1.5,
                        op0=Alu.mult, op1=Alu.add,
                    )
                    nc.vector.tensor_tensor(out=y, in0=y, in1=yt, op=Alu.mult)
                nc.vector.tensor_copy(out=gstat[:G, 1:2], in_=y)

                # broadcast group stats back to channels
                AB = []  # [t] -> [128, 2] (alpha, beta)
                for t in range(2):
                    bc = s_ps.tile([128, 1024], F32, tag="sps", name="bcst")
                    nc.tensor.matmul(out=bc[:, 0:2], lhsT=GMT[:, t], rhs=gstat)
                    bsb = small.tile([128, 2], F32, tag="bsb", name="bsb")
                    nc.vector.tensor_copy(out=bsb, in_=bc[:, 0:2])
                    ab = small.tile([128, 2], F32, tag=f"ab{t}", name=f"ab{t}")
                    # alpha = rstd * w
                    nc.vector.tensor_tensor(
                        out=ab[:, 0:1], in0=bsb[:, 1:2], in1=WN[:, t : t + 1],
                        op=Alu.mult,
                    )
                    # beta = b - mean * alpha
                    nc.vector.tensor_tensor(
                        out=ab[:, 1:2], in0=bsb[:, 0:1], in1=ab[:, 0:1],
                        op=Alu.mult,
                    )
                    nc.vector.tensor_tensor(
                        out=ab[:, 1:2], in0=BN[:, t : t + 1], in1=ab[:, 1:2],
                        op=Alu.subtract,
                    )
                    AB.append(ab)

                # re-warm the PE while the affine assembly finishes on the
                # DVE: the array has been idle since the stats matmuls and
                # would otherwise start K/Q production at the idle clock
                wps2 = s_ps.tile([128, 1024], F32, tag="sps", name="wps2")
                for w in range(4):
                    nc.tensor.matmul(
                        out=wps2[:, 0:512], lhsT=JW, rhs=JR, start=True, stop=True
                    )

                # ---- chunked production helpers ----
                def hq_chunk(c, eng=None):  # normalized queries, 512 cols
                    for t in range(2):
                        (eng or nc.gpsimd).tensor_scalar(
                            out=HQ[t][:, c * 512 : (c + 1) * 512],
                            in0=XQ[t][:, c * 512 : (c + 1) * 512],
                            scalar1=AB[t][:, 0:1], scalar2=AB[t][:, 1:2],
                            op0=Alu.mult, op1=Alu.add,
                        )

                def h_chunk(c):  # normalized keys, 512 cols (gpsimd: pure
                    # SBUF->SBUF, keeps the DVE free for psum drains)
                    for t in range(2):
                        nc.gpsimd.tensor_scalar(
                            out=H[t][:, c * 512 : (c + 1) * 512],
                            in0=X[t][:, c * 512 : (c + 1) * 512],
                            scalar1=AB[t][:, 0:1], scalar2=AB[t][:, 1:2],
                            op0=Alu.mult, op1=Alu.add,
                        )

                def q_chunk(c):  # q projection for queries 512c.. (both t)
                    for t in range(2):
                        ps = s_ps.tile([128, 1024], F32, tag="sps", name="qps")
                        for ct in range(2):
                            nc.tensor.matmul(
                                out=ps[:, 0:512],
                                lhsT=WQ[:, ct, t * 128 : (t + 1) * 128],
                                rhs=HQ[ct][:, c * 512 : (c + 1) * 512],
                                start=(ct == 0), stop=(ct == 1),
                            )
                        nc.vector.tensor_scalar_add(
                            out=QT[t][:, c * 512 : (c + 1) * 512],
                            in0=ps[:, 0:512], scalar1=BQ[:, t : t + 1],
                        )

                def k_piece(n, hp):  # K head-pair hp for keys 512n..
                    ps = s_ps.tile([128, 1024], F32, tag="sps", name="kps")
                    for ct in range(2):
                        nc.tensor.matmul(
                            out=ps[:, 0:512],
                            lhsT=WK[:, ct, hp * 128 : (hp + 1) * 128],
                            rhs=H[ct][:, n * 512 : (n + 1) * 512],
                            start=(ct == 0), stop=(ct == 1),
                        )
                    nc.vector.tensor_scalar_add(
                        out=KZ[hp][:, n * 512 : (n + 1) * 512],
                        in0=ps[:, 0:512], scalar1=BK[:, hp : hp + 1],
                    )

                def v_piece(n, half):  # V^T for key tiles 4n+2*half(+1)
                    j0 = 4 * n + 2 * half
                    ps = s_ps.tile([128, 1024], F32, tag="sps", name="vps")
                    for jo in range(2):
                        for ct in range(2):
                            nc.tensor.matmul(
                                out=ps[:, jo * 512 : jo * 512 + NH * 65],
                                lhsT=H[ct][:, (j0 + jo) * 128 : (j0 + jo + 1) * 128],
                                rhs=WV[:, ct],
                                start=(ct == 0), stop=(ct == 1),
                            )
                        nc.vector.tensor_tensor(
                            out=V[:, j0 + jo],
                            in0=ps[:, jo * 512 : jo * 512 + NH * 65],
                            in1=VB, op=Alu.add,
                        )

                # minimal chain to the first QK: chunk 0 of HQ/H/Q/K.
                # h(0) on gpsimd and hq(0) on the DVE run in parallel.
                h_chunk(0)
                hq_chunk(0, nc.vector)
                k_piece(0, 0)
                q_chunk(0)

                # ic0 production schedule: jp slot -> tasks, per hp.
                # V chunk m lands at slot 2m (just in time for its own AVs),
                # K chunk m+1 and H chunk m+2 at slot 2m+1; hp1 only needs
                # its own K head-pair. q/hq chunks ride along for later ics.
                prod0, prod1 = {}, {}
                prod0[0] = [
                    lambda: h_chunk(1),
                    lambda: v_piece(0, 0), lambda: v_piece(0, 1),
                ]
                for m in range(1, NKC):
                    tasks = []
                    if m + 1 < NKC:
                        tasks.append(lambda c=m + 1: h_chunk(c))
                    tasks.append(lambda c=m: k_piece(c, 0))
                    prod0[2 * m - 1] = tasks
                    prod0[2 * m] = [
                        lambda c=m: v_piece(c, 0), lambda c=m: v_piece(c, 1)
                    ]
                for qi, p in ((1, 4), (2, 8), (3, 12)):
                    prod0[p] = prod0.get(p, []) + [
                        lambda c=qi: hq_chunk(c), lambda c=qi: q_chunk(c)
                    ]
                prod0[14] = prod0.get(14, []) + [lambda: k_piece(0, 1)]
                for m in range(1, NKC):
                    prod1[2 * m - 1] = [lambda c=m: k_piece(c, 1)]

                # exp tile routing across THREE consumers: Act (native Exp),
                # DVE (one-op Schraudolph from psum), gpsimd (Schraudolph from
                # a DMA-staged SBUF copy -- gpsimd cannot read psum; the copy
                # rides the idle sync DMA queue). ic0 leans on Act (DVE does
                # production drains there, gpsimd the h chunks).
                exp_cnt = [0]
                PAT0 = _route_pattern(48, 16)
                PAT = _route_pattern(37, 27)

                def do_exp(S, E):
                    i = exp_cnt[0]
                    exp_cnt[0] += 1
                    lab = (PAT0 if i < 64 else PAT)[i % 64]
                    if lab == "v":
                        nc.vector.tensor_scalar(
                            out=E, in0=S, scalar1=A16 * SCALE, scalar2=B16,
                            op0=Alu.mult, op1=Alu.add,
                        )
                    else:
                        nc.scalar.activation(
                            out=E.bitcast(BF16), in_=S, func=Exp, scale=SCALE
                        )

                # ---- attention + projection (oT form: queries on psum
                # partitions, exp(scores) streamed as the stationary operand,
                # per-partition softmax normalization).
                # AV matmuls and per-block finishers (normalize, transpose,
                # proj, residual) are DEFERRED through a global work queue so
                # the next block's QK/exp stream is emitted ahead of them --
                # the in-order engines never serialize at hp/ic boundaries.
                pend = []      # (emit_av_closure, block_key)
                finishers = {}  # block_key -> closure run after its last AV

                def pump(k):
                    for _ in range(k):
                        if not pend:
                            break
                        fn, key = pend.pop(0)
                        fn()
                        if key in finishers and not any(
                            k2 == key for _, k2 in pend
                        ):
                            finishers.pop(key)()

                for ic in range(NIC):
                    oTn = att.tile([128, 4, 256], BF16, tag="oTn", name="oTn")
                    OSB = att.tile([128, 2, 512], BF16, tag="osb", name="osb")
                    for hp in range(2):
                        oT = [
                            acc_ps.tile(
                                [128, 4, 68], F32, tag=f"ot{h2}", name=f"ot{h2}"
                            )
                            for h2 in range(2)
                        ]

                        def qk_into(S, j, hp=hp, ic=ic):
                            for h2 in range(2):
                                nc.tensor.matmul(
                                    out=S[:, h2 * 512 : (h2 + 1) * 512],
                                    lhsT=KZ[hp][
                                        h2 * 64 : (h2 + 1) * 64,
                                        j * 128 : (j + 1) * 128,
                                    ],
                                    rhs=QT[hp][
                                        h2 * 64 : (h2 + 1) * 64,
                                        ic * 512 : (ic + 1) * 512,
                                    ],
                                    start=True, stop=True,
                                )

                        def av_from(E, j, oT=oT, hp=hp):
                            for h2 in range(2):
                                head = 2 * hp + h2
                                for isub in range(4):
                                    nc.tensor.matmul(
                                        out=oT[h2][:, isub, 0:65],
                                        lhsT=E[
                                            :,
                                            h2 * 512 + isub * 128 : h2 * 512
                                            + (isub + 1) * 128,
                                        ].bitcast(BF16),
                                        rhs=V[:, j, head * 65 : (head + 1) * 65],
                                        start=(j == 0 and isub == 0),
                                        stop=(j == NJT - 1 and isub == 3),
                                    )

                        for jp in range(NJT // 2):
                            if ic == 0:
                                sched = prod0 if hp == 0 else prod1
                                for task in sched.get(jp, ()):
                                    task()
                            for jo in range(2):
                                j = 2 * jp + jo
                                S = s_ps.tile(
                                    [128, 1024], F32, tag="sps", name="s"
                                )
                                qk_into(S, j)
                                E = expp.tile(
                                    [128, 1024], I16, tag="exps", name="e"
                                )
                                do_exp(S, E)
                                pend.append(
                                    (lambda E=E, j=j, f=av_from: f(E, j), (ic, hp))
                                )
                            if len(pend) > 2 * LAGP:
                                pump(2)

                        def mk_norm(oT=oT, hp=hp, oTn=oTn):
                            def fin():
                                # normalize by the ones-column sums (per-
                                # partition; one strided reciprocal covers all
                                # 4 sub-tiles)
                                for h2 in range(2):
                                    head = 2 * hp + h2
                                    r4 = small.tile(
                                        [128, 4], F32, tag="recip", name="recip"
                                    )
                                    nc.vector.reciprocal(
                                        out=r4,
                                        in_=oT[h2][:, :, 64:65].rearrange(
                                            "p a b -> p (a b)"
                                        ),
                                    )
                                    for isub in range(4):
                                        nc.vector.tensor_scalar_mul(
                                            out=oTn[
                                                :, isub, head * 64 : (head + 1) * 64
                                            ],
                                            in0=oT[h2][:, isub, 0:64],
                                            scalar1=r4[:, isub : isub + 1],
                                        )
                            return fin

                        finishers[(ic, hp)] = mk_norm()

                    def mk_boundary(norm1=finishers[(ic, 1)], oTn=oTn, OSB=OSB, ic=ic):
                        def fin():
                            norm1()
                            # transpose oTn -> [channels, 512 queries]: all 8
                            # land in one score-pool psum bank (chained group)
                            tps = s_ps.tile([128, 1024], F32, tag="sps", name="tps")
                            tpb = tps.bitcast(BF16)  # [128, 2048]
                            for ct in range(2):
                                for isub in range(4):
                                    k8 = ct * 4 + isub
                                    nc.tensor.matmul(
                                        tpb[:, k8 * 128 : (k8 + 1) * 128],
                                        oTn[:, isub, ct * 128 : (ct + 1) * 128],
                                        IDENT,
                                        is_transpose=True,
                                        start=(k8 == 0), stop=(k8 == 7),
                                    )
                            for ct in range(2):
                                nc.vector.tensor_copy(
                                    out=OSB[:, ct],
                                    in_=tpb[:, ct * 512 : (ct + 1) * 512],
                                )
                            # proj + bias + residual
                            pj = s_ps.tile([128, 1024], F32, tag="sps", name="pj")
                            for mt in range(2):
                                for ct in range(2):
                                    nc.tensor.matmul(
                                        out=pj[:, mt * 512 : (mt + 1) * 512],
                                        lhsT=WP[:, ct, mt * 128 : (mt + 1) * 128],
                                        rhs=OSB[:, ct],
                                        start=(ct == 0), stop=(ct == 1),
                                    )
                                ob = att.tile(
                                    [128, 512], F32, tag="outsb", name="outsb"
                                )
                                nc.vector.scalar_tensor_tensor(
                                    out=ob, in0=pj[:, mt * 512 : (mt + 1) * 512],
                                    scalar=BP[:, mt : mt + 1],
                                    in1=XQ[mt][:, ic * 512 : (ic + 1) * 512],
                                    op0=Alu.add, op1=Alu.add,
                                )
                                nc.sync.dma_start(
                                    out=out[
                                        mt * 128 : (mt + 1) * 128,
                                        ic * 512 : (ic + 1) * 512,
                                    ],
                                    in_=ob,
                                )
                        return fin

                    finishers[(ic, 1)] = mk_boundary()
                pump(len(pend))
    if finalize:
        nc.finalize()
    return nc


def _prep_weights(norm_w, norm_b, qkv_w, qkv_b, proj_w, proj_b):
    """Host-side layout (pure reshapes/transposes + dtype casts of weights)."""
    import ml_dtypes

    f = np.float32
    cdt = ml_dtypes.bfloat16

    def ctile(v):  # (256,) -> (128, 2) per channel-tile columns
        return np.ascontiguousarray(np.asarray(v).reshape(2, 128).T, dtype=f)

    def ptile(m):  # (256, N) -> (128, 2, N)
        return np.ascontiguousarray(
            np.asarray(m).reshape(2, 128, -1).transpose(1, 0, 2), dtype=f
        )

    qkv_w = np.asarray(qkv_w)
    qkv_b = np.asarray(qkv_b)
    wqT = qkv_w[:C].T  # (256, 256)
    wkT = qkv_w[C : 2 * C].T  # (256, 256): out col o = head-pair*128 + row
    wvm = qkv_w[2 * C :]  # (256, 256)
    wvT = np.zeros((C, NH * 65), dtype=f)
    vb = np.zeros((128, NH * 65), dtype=f)
    for h in range(NH):
        wvT[:, h * 65 : h * 65 + 64] = wvm[h * 64 : (h + 1) * 64].T
        vb[:, h * 65 : h * 65 + 64] = qkv_b[
            2 * C + h * 64 : 2 * C + (h + 1) * 64
        ][None, :]
        vb[:, h * 65 + 64] = 1.0  # ones column -> denominator at oT column 64
    # zero-padded group masks (value 1/32 for group-mean aggregation; one-hot
    # transpose for the broadcast back to channels)
    gm = np.zeros((C, 128), dtype=f)
    for c in range(C):
        gm[c, c // 32] = 1.0 / 32.0
    gmaskT = np.zeros((128, 2, 128), dtype=f)
    for c in range(C):
        gmaskT[c // 32, c // 128, c % 128] = 1.0

    return dict(
        wn2=ctile(norm_w),
        bn2=ctile(norm_b),
        wq=ptile(wqT).astype(cdt),
        bq2=ctile(qkv_b[:C]),
        wk=ptile(wkT).astype(cdt),
        bk2=ctile(qkv_b[C : 2 * C]),
        wv=ptile(wvT).astype(cdt),
        vb=vb,
        wproj=ptile(np.asarray(proj_w).T).astype(cdt),
        ident=np.eye(128, dtype=cdt),
        bproj2=ctile(proj_b),
        gmask=ptile(gm).astype(cdt),
        gmaskT=gmaskT.astype(cdt),
    )


_NC_CACHE = {}
_RUN_OPTS = {}  # extra kwargs for run_bass_kernel_spmd (test harness sets trace)
LAST_RESULT = None


def _get_nc():
    if "nc" not in _NC_CACHE:
        _NC_CACHE["nc"] = build()
    return _NC_CACHE["nc"]


def kernel(x, norm_w, norm_b, qkv_w, qkv_b, proj_w, proj_b, **_):
    import ml_dtypes

    nc = _get_nc()
    w = _prep_weights(norm_w, norm_b, qkv_w, qkv_b, proj_w, proj_b)
    x = np.asarray(x, dtype=np.float32)
    Bv, Cv, Hv, Wv = x.shape
    xf = x.reshape(Bv, Cv, Hv * Wv)
    xb = xf.astype(ml_dtypes.bfloat16)
    in_maps = []
    for j in range(8):
        b, qh = j // 2, j % 2
        m = dict(w)
        m["x"] = np.ascontiguousarray(xb[b])
        m["xq"] = np.ascontiguousarray(xb[b][:, qh * Q : (qh + 1) * Q])
        in_maps.append(m)
    res = run_bass_kernel_spmd(nc, in_maps, core_ids=list(range(8)), **_RUN_OPTS)
    global LAST_RESULT
    LAST_RESULT = res
    outf = np.empty((Bv, Cv, Hv * Wv), dtype=np.float32)
    for j in range(8):
        b, qh = j // 2, j % 2
        outf[b][:, qh * Q : (qh + 1) * Q] = res.results[j]["out"]
    return outf.reshape(Bv, Cv, Hv, Wv)
